# revision 21
# baseline (speedup 1.0000x reference)
"""Trainium2 Bass kernel for nn_CurvatureReg:
mean(tv_min(curvature(gauss(pred))) * dilate_mask(pred)).

v2: bf16 pipeline on centered inputs (pred-0.5), all H-direction stencils as
banded PE matmuls (H-blur, Gy, ddy, |dH|, boxH) so no partition-shift DMAs,
eps folded into the Sqrt activation bias, DVE ops in bf16 2x mode.

Sharding: 16 logical shards = 2 batches x 4 D-chunks x 2 H-halves; two shards
run sequentially per core. Per-shard layout: partitions = H (94 = 80 owned +
7 halo), free = (D_local 54, W). Host sums per-partition partials over owned
rows. The compiled PJRT executable is cached across kernel() calls.
"""
import os
import numpy as np
import ml_dtypes

import concourse.bacc as bacc
import concourse.bass as bass
import concourse.mybir as mybir
import concourse.tile as tile

dt = mybir.dt
F32 = dt.float32
BF16 = dt.bfloat16
ALU = mybir.AluOpType
ACTF = mybir.ActivationFunctionType
BF = ml_dtypes.bfloat16

_NCORES = int(os.environ.get("KCORES", "8"))
_REPEAT = int(os.environ.get("KREPEAT", "1"))

NB, DIM = 2, 160
CHUNK = 40                # owned D planes per core
HALO = 7
DEXT = CHUNK + 2 * HALO   # 54
WPAD = DIM + 8
HOWN = 80                 # owned H rows per shard
HEXT = HOWN + 2 * HALO    # 94 partitions
THRES_C = 0.1 - 0.5       # threshold on centered pred
EPS = 1e-6
DELTA = 16.0 * EPS * EPS  # sqrt(4*s + DELTA) ~= 2*sqrt(s) + 4*EPS at s=0
CH = 3                    # D-rows per PE chunk (3*156 <= 512 PSUM f32)


class Field:
    def __init__(self, t, d_base, w_base):
        self.t = t
        self.d0 = d_base
        self.w0 = w_base

    def ap(self, d_lo, d_hi, w_lo, w_hi, p0=0, p1=HEXT):
        return self.t[p0:p1, d_lo - self.d0:d_hi - self.d0,
                      w_lo - self.w0:w_hi - self.w0]


def _build(kd, kw):
    nc = bacc.Bacc(None, target_bir_lowering=False, debug=False)

    ins = {}
    for h in (0, 1):
        ins[f"slab{h}"] = nc.dram_tensor(f"slab{h}", [HEXT, DEXT, WPAD], BF16,
                                         kind="ExternalInput").ap()
    ins["bb"] = nc.dram_tensor("bb", [HEXT, HEXT], BF16,
                               kind="ExternalInput").ap()
    ins["cc"] = nc.dram_tensor("cc", [HEXT, HEXT], BF16,
                               kind="ExternalInput").ap()
    ins["m3"] = nc.dram_tensor("m3", [HEXT, HEXT], BF16,
                               kind="ExternalInput").ap()
    dmask_d = nc.dram_tensor("dmask", [HEXT, 16], BF16, kind="ExternalInput").ap()
    part_d = [nc.dram_tensor(f"partial{h}", [HEXT, 1], F32,
                             kind="ExternalOutput").ap() for h in (0, 1)]

    P = HEXT
    with tile.TileContext(nc) as tc:
        with tc.tile_pool(name="big", bufs=1) as pool, \
             tc.tile_pool(name="ps", bufs=2, space="PSUM") as psp:

            def mk(rows, cols, dtp, name):
                return pool.tile([P, rows, cols], dtp, tag=name, name=name)

            S1 = mk(DEXT, WPAD, BF16, "S1")   # slab (w base -4)
            S2 = mk(DEXT, 156, BF16, "S2")    # A (W-blur) -> phi
            S3 = mk(46, 156, BF16, "S3")      # Bf (D-blur)
            S5 = mk(44, 154, BF16, "S5")      # Gx -> nx -> dD
            S6 = mk(44, 154, BF16, "S6")      # Gy -> ny -> dW -> prod
            S7 = mk(44, 154, BF16, "S7")      # Gz -> nz -> |dH|
            S8 = mk(44, 154, F32, "S8")       # s32 -> r32
            S9 = mk(44, 154, BF16, "S9")      # sq scratch -> ddx -> k
            S10 = mk(44, 154, BF16, "S10")    # sq scratch -> rbf -> ddz -> ddy
            SM1 = mk(42, 152, BF16, "SM1")    # b -> boxH -> maskf
            SM2 = mk(42, 152, BF16, "SM2")    # boxW -> boxD
            bb_t = pool.tile([P, HEXT], BF16, tag="bb", name="bb")
            cc_t = pool.tile([P, HEXT], BF16, tag="cc", name="cc")
            m3_t = pool.tile([P, HEXT], BF16, tag="m3", name="m3")
            dmask_t = pool.tile([P, 16], BF16, tag="dmaskt", name="dmaskt")
            pacc = [pool.tile([P, 1], F32, tag=f"pacc{h}", name=f"pacc{h}")
                    for h in (0, 1)]
            delta_t = pool.tile([P, 1], F32, tag="delta", name="delta")
            nc.vector.memset(delta_t, DELTA)

            pred = Field(S1, 0, -4)
            A = Field(S2, 0, 2)
            Bf = Field(S3, 4, 2)
            phi = Field(S2, 4, 2)
            Gx = Field(S5, 5, 3)
            Gy = Field(S6, 5, 3)
            Gz = Field(S7, 5, 3)
            s32 = Field(S8, 5, 3)
            sqA = Field(S9, 5, 3)
            sqB = Field(S10, 5, 3)
            rbf = Field(S10, 5, 3)
            ddx = Field(S9, 6, 4)     # -> k
            ddz = Field(S10, 6, 4)
            ddy = Field(S10, 6, 3)
            dD = Field(S5, 7, 5)
            dW = Field(S6, 7, 5)
            dH = Field(SM2, 7, 5)
            prod = Field(S7, 7, 5)
            b = Field(SM1, 6, 4)
            boxW = Field(SM2, 6, 4)
            boxH = Field(SM1, 6, 4)
            boxD = Field(SM2, 7, 5)
            maskf = Field(SM1, 7, 5)

            def tt(dst, a, b_, op, region, p1=HEXT):
                d0, d1, w0, w1 = region
                nc.vector.tensor_tensor(dst.ap(d0, d1, w0, w1, 0, p1),
                                        a.ap(d0, d1, w0, w1, 0, p1),
                                        b_.ap(d0, d1, w0, w1, 0, p1), op)

            def tt_shift(dst, a, b_, op, region, da=0, wa=0, db=0, wb=0):
                d0, d1, w0, w1 = region
                nc.vector.tensor_tensor(
                    dst.ap(d0, d1, w0, w1),
                    a.ap(d0 + da, d1 + da, w0 + wa, w1 + wa),
                    b_.ap(d0 + db, d1 + db, w0 + wb, w1 + wb), op)

            def blur_free(dst, src, taps, region, axis):
                d0, d1, w0, w1 = region
                dst_ap = dst.ap(d0, d1, w0, w1)
                for i, k in enumerate(range(-4, 5)):
                    da, wa = (k, 0) if axis == "d" else (0, k)
                    src_ap = src.ap(d0 + da, d1 + da, w0 + wa, w1 + wa)
                    if i == 0:
                        nc.scalar.mul(dst_ap, src_ap, float(taps[i]))
                    else:
                        nc.vector.scalar_tensor_tensor(
                            dst_ap, src_ap, float(taps[i]), dst_ap,
                            ALU.mult, ALU.add)

            def pe_band(dst, src, stat, region, evac):
                """out[h] = sum_j stat[j,h]*src[j] (banded matmul over
                partitions), chunked by CH d-rows, grouped PSUM evacuation.
                evac: 'copy' or 'abs' (Abs fused into the PSUM->SBUF move).
                Requires wn == dst tile row length (full-width region) so
                grouped chunks evacuate as one flat AP."""
                d0, d1, w0, w1 = region
                wn = w1 - w0
                assert dst.t.shape[2] == wn and w0 - dst.w0 == 0, \
                    "pe_band needs full-width dst"
                chunks = []
                c0 = d0
                while c0 < d1:
                    chunks.append((c0, min(c0 + CH, d1)))
                    c0 = min(c0 + CH, d1)
                for g in range(0, len(chunks), 4):
                    grp = chunks[g:g + 4]
                    ps = psp.tile([P, 4, 512], F32, tag="ps", name="ps")
                    for ci, (a0, a1) in enumerate(grp):
                        nc.tensor.matmul(ps[:, ci:ci + 1, 0:(a1 - a0) * wn],
                                         stat, src.ap(a0, a1, w0, w1),
                                         start=True, stop=True)
                    full = [c for c in grp if c[1] - c[0] == CH]
                    if full:
                        nfull = len(full)
                        pin = ps[0:P, 0:nfull, 0:CH * wn]
                        dbase = dst.ap(full[0][0], full[-1][1], w0, w1)
                        dout = bass.AP(tensor=dbase.tensor, offset=dbase.offset,
                                       ap=[dbase.ap[0], [CH * wn, nfull],
                                           [1, CH * wn]])
                        if evac == "abs":
                            nc.scalar.activation(dout, pin, ACTF.Abs)
                        else:
                            nc.scalar.copy(dout, pin)
                    for ci, (a0, a1) in enumerate(grp[len(full):], len(full)):
                        dst_ap = dst.ap(a0, a1, w0, w1)
                        pin = ps[:, ci:ci + 1, 0:(a1 - a0) * wn]
                        if evac == "abs":
                            nc.scalar.activation(dst_ap, pin, ACTF.Abs)
                        else:
                            nc.scalar.copy(dst_ap, pin)

            def shard(h):
                nc.sync.dma_start(out=S1, in_=ins[f"slab{h}"])
                if h == 0:
                    nc.sync.dma_start(out=bb_t, in_=ins["bb"])
                    nc.sync.dma_start(out=cc_t, in_=ins["cc"])
                    nc.sync.dma_start(out=m3_t, in_=ins["m3"])
                    nc.sync.dma_start(out=dmask_t, in_=dmask_d)

                # ---- mask pipeline (bf16) ----
                nc.vector.tensor_scalar(b.ap(6, 48, 4, 156),
                                        pred.ap(6, 48, 4, 156),
                                        THRES_C, None, ALU.is_ge)
                MB = (6, 48, 5, 155)
                tt_shift(boxW, b, b, ALU.add, MB, wa=-1, wb=1)
                tt(boxW, boxW, b, ALU.add, MB)
                pe_band(boxH, boxW, m3_t, (6, 48, 4, 156), "copy")
                VD = (7, 47, 5, 155)
                tt_shift(boxD, boxH, boxH, ALU.add, VD, da=-1, db=1)
                tt(boxD, boxD, boxH, ALU.add, VD)
                nc.vector.tensor_scalar(maskf.ap(*VD), boxD.ap(*VD),
                                        0.5, None, ALU.is_ge)
                # zero global-D borders (cols 0-9) and H borders (col 10+h)
                base = SM1[:, 0:5, 0:150]
                m_ap = bass.AP(tensor=base.tensor, offset=base.offset,
                               ap=[base.ap[0], [35 * 152, 2], [152, 5], [1, 150]])
                dbase = dmask_t[:, 0:1]
                dm_ap = bass.AP(tensor=dbase.tensor, offset=dbase.offset,
                                ap=[dbase.ap[0], [5, 2], [1, 5], [0, 150]])
                nc.vector.tensor_tensor(m_ap, m_ap, dm_ap, ALU.mult)
                hbase = dmask_t[:, 10 + h:11 + h]
                hm_ap = bass.AP(tensor=hbase.tensor, offset=hbase.offset,
                                ap=[hbase.ap[0], [0, 40], [0, 150]])
                mf = maskf.ap(*VD)
                nc.vector.tensor_tensor(mf, mf, hm_ap, ALU.mult)

                # ---- separable Gaussian: W (DVE), D (DVE), H (PE) ----
                blur_free(A, pred, kw, (0, 54, 2, 158), "w")
                blur_free(Bf, A, kd, (4, 50, 2, 158), "d")
                pe_band(phi, Bf, bb_t, (4, 50, 2, 158), "copy")

                # ---- gradients (doubled) ----
                R = (5, 49, 3, 157)
                tt_shift(Gx, phi, phi, ALU.subtract, R, da=1, db=-1)
                pe_band(Gy, phi, cc_t, R, "copy")
                tt_shift(Gz, phi, phi, ALU.subtract, R, wa=1, wb=-1)

                # s = Gx^2+Gy^2+Gz^2 (bf16), r = 1/sqrt(4*s+DELTA) (f32)
                tt(sqA, Gx, Gx, ALU.mult, R)
                tt(sqB, Gy, Gy, ALU.mult, R)
                tt(sqA, sqA, sqB, ALU.add, R)
                tt(sqB, Gz, Gz, ALU.mult, R)
                tt(sqA, sqA, sqB, ALU.add, R)
                nc.scalar.activation(s32.ap(*R), sqA.ap(*R),
                                     ACTF.Sqrt, bias=delta_t[:, 0:1], scale=4.0)
                nc.vector.reciprocal(s32.ap(*R), s32.ap(*R))
                nc.vector.tensor_scalar(rbf.ap(*R), s32.ap(*R), 1.0,
                                        None, ALU.mult)
                tt(Gx, Gx, rbf, ALU.mult, R)
                tt(Gy, Gy, rbf, ALU.mult, R)
                tt(Gz, Gz, rbf, ALU.mult, R)

                # ---- curvature k = div n (n doubled-normalized: k exact) ----
                K = (6, 48, 4, 156)
                tt_shift(ddx, Gx, Gx, ALU.subtract, K, da=1, db=-1)
                tt_shift(ddz, Gz, Gz, ALU.subtract, K, wa=1, wb=-1)
                tt(ddx, ddx, ddz, ALU.add, K)
                pe_band(ddy, Gy, cc_t, (6, 48, 3, 157), "copy")
                tt(ddx, ddx, ddy, ALU.add, K)
                k = ddx

                # ---- tv-min ----
                V = (7, 47, 5, 155)
                tt_shift(dD, k, k, ALU.subtract, V, da=1, db=-1)
                nc.scalar.activation(dD.ap(*V), dD.ap(*V), ACTF.Abs)
                tt_shift(dW, k, k, ALU.subtract, V, wa=1, wb=-1)
                nc.scalar.activation(dW.ap(*V), dW.ap(*V), ACTF.Abs)
                pe_band(dH, k, cc_t, (7, 47, 5, 157), "abs")
                tt(dD, dD, dW, ALU.min, V)
                tt(dD, dD, dH, ALU.min, V)

                # ---- masked sum ----
                nc.vector.scalar_tensor_tensor(
                    prod.ap(*V), dD.ap(*V), 1.0, maskf.ap(*V),
                    ALU.mult, ALU.mult, accum_out=pacc[h])
                nc.sync.dma_start(out=part_d[h], in_=pacc[h])

            if _REPEAT > 1:
                with tc.For_i(0, _REPEAT) as _:
                    shard(0)
                    shard(1)
            else:
                shard(0)
                shard(1)

    nc.compile()
    return nc


_PROG = None
_RUNNER = None


def _taps_from_gk(gk):
    g = np.asarray(gk, dtype=np.float64).reshape(9, 9, 9)
    return g.sum(axis=(1, 2)), g.sum(axis=(0, 2)), g.sum(axis=(0, 1))


def _band_blur(kh):
    # Full (unclipped) band on the 94-row shard window; pad rows carry -0.5
    # in the slab so the centered blur is exact at global edges.
    r = np.arange(HEXT)
    diff = r[:, None] - r[None, :]
    bblur = np.zeros((HEXT, HEXT), np.float64)
    m = np.abs(diff) <= 4
    bblur[m] = kh[(diff + 4)[m]]
    return bblur.astype(BF)


def _cdiff_band():
    # lhsT[j, h]: out[h] = src[h+1] - src[h-1]
    C = np.zeros((HEXT, HEXT), np.float32)
    i = np.arange(HEXT - 1)
    C[i + 1, i] = 1.0
    C[i, i + 1] = -1.0
    return C.astype(BF)


def _m3_band():
    M = np.zeros((HEXT, HEXT), np.float32)
    i = np.arange(HEXT)
    M[i, i] = 1.0
    j = np.arange(HEXT - 1)
    M[j + 1, j] = 1.0
    M[j, j + 1] = 1.0
    return M.astype(BF)


def _in_maps(pred, gk):
    pred = np.asarray(pred, dtype=np.float32)
    kd, kh, kw = _taps_from_gk(gk)
    bblur = _band_blur(kh)
    cc = _cdiff_band()
    m3 = _m3_band()
    in_maps = []
    for core in range(8):
        n, c = divmod(core, 4)
        gd0 = c * CHUNK - HALO
        dlo, dhi = max(gd0, 0), min(gd0 + DEXT, DIM)
        m = {"bb": bblur, "cc": cc, "m3": m3}
        dmask = np.ones((HEXT, 16), np.float32)
        if c == 0:
            dmask[:, 0:5] = 0.0
        if c == 3:
            dmask[:, 5:10] = 0.0
        for h in (0, 1):
            h0 = h * HOWN - HALO
            hlo, hhi = max(h0, 0), min(h0 + HEXT, DIM)
            slab = np.full((HEXT, DEXT, WPAD), -0.5, BF)
            block = pred[n, 0, dlo:dhi, hlo:hhi, :]      # (d, h, w)
            slab[hlo - h0:hhi - h0, dlo - gd0:dhi - gd0, 4:4 + DIM] = \
                (block.transpose(1, 0, 2) - np.float32(0.5)).astype(BF)
            m[f"slab{h}"] = slab
            hg = h0 + np.arange(HEXT)
            dmask[:, 10 + h] = ((hg >= 5) & (hg < DIM - 5)).astype(np.float32)
        m["dmask"] = dmask.astype(BF)
        in_maps.append(m)
    return in_maps


def _make_runner(nc, n_cores):
    """Build a cached PJRT executable for nc (axon path), once."""
    import jax
    from jax.sharding import Mesh, PartitionSpec
    from jax.experimental.shard_map import shard_map
    from concourse import bass2jax

    bass2jax.install_neuronx_cc_hook()

    partition_name = (nc.partition_id_tensor.name
                      if nc.partition_id_tensor else None)
    in_names, out_names, out_avals, zero_outs = [], [], [], []
    for alloc in nc.m.functions[0].allocations:
        if not isinstance(alloc, mybir.MemoryLocationSet):
            continue
        name = alloc.memorylocations[0].name
        if alloc.kind == "ExternalInput":
            if name != partition_name:
                in_names.append(name)
        elif alloc.kind == "ExternalOutput":
            out_names.append(name)
            shape = tuple(alloc.tensor_shape)
            dtype = mybir.dt.np(alloc.dtype)
            out_avals.append(jax.core.ShapedArray(shape, dtype))
            zero_outs.append(np.zeros(shape, dtype))
    n_params = len(in_names)
    n_outs = len(out_avals)
    all_names = in_names + out_names
    if partition_name is not None:
        all_names = all_names + [partition_name]
    donate = tuple(range(n_params, n_params + n_outs))

    def _body(*args):
        operands = list(args)
        if partition_name is not None:
            operands.append(bass2jax.partition_id_tensor())
        outs = bass2jax._bass_exec_p.bind(
            *operands,
            out_avals=tuple(out_avals),
            in_names=tuple(all_names),
            out_names=tuple(out_names),
            lowering_input_output_aliases=(),
            sim_require_finite=True,
            sim_require_nnan=True,
            nc=nc,
        )
        return tuple(outs)

    devices = jax.devices()[:n_cores]
    mesh = Mesh(np.asarray(devices), ("core",))
    in_specs = (PartitionSpec("core"),) * (n_params + n_outs)
    out_specs = (PartitionSpec("core"),) * n_outs
    sharded = jax.jit(
        shard_map(_body, mesh=mesh, in_specs=in_specs, out_specs=out_specs,
                  check_rep=False),
        donate_argnums=donate, keep_unused=True)

    def run(in_maps):
        concat_in = [
            np.concatenate([np.asarray(in_maps[c][nm]) for c in range(n_cores)],
                           axis=0)
            for nm in in_names
        ]
        concat_zeros = [
            np.zeros((n_cores * z.shape[0], *z.shape[1:]), z.dtype)
            for z in zero_outs
        ]
        out_arrs = sharded(*concat_in, *concat_zeros)
        return [
            {nm: np.asarray(out_arrs[i]).reshape(n_cores, *out_avals[i].shape)[c]
             for i, nm in enumerate(out_names)}
            for c in range(n_cores)
        ]

    return run


def kernel(pred, gk):
    global _PROG, _RUNNER
    gk = np.asarray(gk, dtype=np.float32)
    kd, kh, kw = _taps_from_gk(gk)

    key = (tuple(kd), tuple(kw))
    if _PROG is None or _PROG[1] != key:
        _PROG = (_build(kd, kw), key)
        _RUNNER = _make_runner(_PROG[0], _NCORES)

    in_maps = _in_maps(pred, gk)
    res = _RUNNER(in_maps[:_NCORES])
    total = 0.0
    for core in range(_NCORES):
        for h in (0, 1):
            p = np.asarray(res[core][f"partial{h}"], dtype=np.float64)
            total += p[HALO:HALO + HOWN].sum()
    return np.float32(total / (NB * DIM * DIM * DIM))
